# revision 1
# baseline (speedup 1.0000x reference)
"""BSC loss (single label) on 8 Trainium2 NeuronCores.

Reference computation (B=8192, H=256, C=32, T=0.1):
    f   = l2_normalize(features)                      # [B, H]
    sim = f @ f.T / T                                 # [B, B] (never materialized here)
    E   = exp(sim) with zeroed diagonal
    class_sum[i, c] = sum_{j: label_j = c} E[i, j]
    counts_excl[i, c] = counts[c] - onehot[i, c]
    denom_i = sum_c where(ce > 0, class_sum / max(ce, 1))
    mean_pos_sim_i = (sum_{j != i, same label} sim[i, j]) / P_i,  P_i = counts[l_i] - 1
    loss_i = log(max(denom_i, 1e-30)) - mean_pos_sim_i   (if P_i > 0)
    loss = sum(loss_i) / n_valid

Distribution: each core gets the inputs ROTATED by core*1024 rows and computes
the partial (sum loss_i, n_valid) over rotated rows 0..1023 (its anchor shard).
Rotation makes the program identical across cores (pure SPMD, static
addresses): anchors are always columns 0..1023 of the similarity slab and the
self-pair diagonal always falls in key blocks 0..7 at a fixed offset.

Key structure per core:
  stage A: normalize rows chunk-wise, build fT [256, 8192] (bf16) via PE
           transposes, onehot [128, 64*32] (bf16), and
           g_ext = onehot.T @ [f_norm | 1]  [32, 257] in PSUM
           (g = per-class feature sums; last column = exact class counts).
  hot loop over 64 key blocks: sim slab [128 keys, 1024 anchors] in PSUM via
           bf16 matmuls, exp(10*sim) on ACT into SBUF (bf16), zero the
           diagonal for key blocks 0..7, then accumulate class_sum [32, 1024]
           with a onehot.T @ E matmul.
  positives term needs no B^2 pass: sum_{j: label_j=c} sim[i,j] = (g @ f.T)/T
           because sim is linear in the key features.
  finale: [32, 1024] vector math + ones-vector matmul partition reductions.

The scalar partials are summed on the host (8 pairs).
"""

import numpy as np

import bass_rust
import concourse.bass as bass
import concourse.tile as tile
from concourse import mybir
from concourse.bass_utils import run_bass_kernel_spmd

F32 = mybir.dt.float32
BF16 = mybir.dt.bfloat16

B = 8192
H = 256
C = 32
N_CORES = 8
SHARD = B // N_CORES          # 1024 anchors per core
N_CHUNKS = B // 128           # 64 row chunks / key blocks
TEMP_INV = 10.0               # 1 / temperature


class SplitDrainTileContext(tile.TileContext):
    """TileContext that caps sem waits at one per instruction.

    The walrus build in this container rejects instructions carrying more
    than one sync wait ("Too many sync wait commands", e.g. on Drain and
    TensorScalarPtr). Tile freely attaches several waits per instruction, so
    split the surplus onto same-engine nops inserted immediately before the
    instruction (identical semantics: the engine blocks on every wait before
    executing it).
    """

    MAX_DRAIN_WAITS = 1

    def _lower_ordered_insts(self, ordered):
        for insts in ordered.values():
            new_list = []
            for inst in insts:
                si = inst.sync_info
                ws = list(si.on_wait) if si is not None and si.on_wait else []
                if len(ws) > 1:
                    for k, w in enumerate(ws[:-1]):
                        new_list.append(mybir.InstNoOp(
                            name=f"{inst.name}_sw{k}",
                            engine=inst.engine,
                            bass_nofuse=True,
                            sync_info=mybir.SyncInfo(on_wait=[w], on_update=[]),
                        ))
                    inst.sync_info = mybir.SyncInfo(
                        on_wait=[ws[-1]], on_update=list(si.on_update or []))
                new_list.append(inst)
            insts[:] = new_list
        super()._lower_ordered_insts(ordered)

    def _drain_and_barrier(self, tick_clock, wait_clock):
        probe = self.nc.sync.nop()
        wait_clock.add_sem_waits(
            probe.ins, bass_rust.ScopedClock({None: tick_clock.global_clock})
        )
        si = probe.ins.sync_info
        waits = list(si.on_wait) if si is not None and si.on_wait else []
        probe.ins.sync_info = bass_rust.SyncInfo(
            on_wait=waits[: self.MAX_DRAIN_WAITS], on_update=[]
        )
        for i in range(self.MAX_DRAIN_WAITS, len(waits), self.MAX_DRAIN_WAITS):
            n = self.nc.sync.nop()
            n.ins.sync_info = bass_rust.SyncInfo(
                on_wait=waits[i : i + self.MAX_DRAIN_WAITS], on_update=[]
            )
        self.nc.sync.drain()

        self.nc.all_engine_barrier()
        assert self.sems is not None
        popped = self.nc._tile_sem_poison_stack.pop()
        assert popped is self._sem_poison
        self.nc.clear_and_free_semaphores(list(self.sems.allocated().values()))
        self.nc.all_engine_barrier()


def build_program(n_iters: int = 1):
    """Emit the SPMD program. n_iters > 1 wraps the body in a hardware loop
    (identical recompute) for wall-clock timing runs."""
    nc = bass.Bass("TRN2", target_bir_lowering=False, debug=False,
                   num_devices=N_CORES)

    feat = nc.dram_tensor("feat", [B, H], F32, kind="ExternalInput")
    lab = nc.dram_tensor("lab", [128, N_CHUNKS], F32, kind="ExternalInput")
    out = nc.dram_tensor("partials", [1, 2], F32, kind="ExternalOutput")

    with SplitDrainTileContext(nc) as tc:
        if n_iters == 1:
            emit_body(nc, tc, feat, lab, out)
        else:
            hints = (mybir.EngineType.PE, mybir.EngineType.Activation,
                     mybir.EngineType.DVE, mybir.EngineType.SP,
                     mybir.EngineType.Pool)
            with tc.For_i(0, n_iters, 1, hint_engines=hints):
                emit_body(nc, tc, feat, lab, out)
    return nc


def emit_body(nc, tc, feat, lab, out):
    from contextlib import ExitStack

    ACT = mybir.ActivationFunctionType
    OP = mybir.AluOpType
    AX = mybir.AxisListType

    with ExitStack() as ctx:
        ep = ctx.enter_context  # shorthand

        # ---- persistent SBUF ----
        const_pool = ep(tc.tile_pool(name="consts", bufs=1))
        identity = const_pool.tile([128, 128], BF16)
        from concourse import masks
        masks.make_identity(nc, identity[:])
        diagmask = const_pool.tile([128, 128], BF16)
        nc.gpsimd.memset(diagmask[:], 1.0)
        nc.gpsimd.affine_select(
            out=diagmask[:], in_=diagmask[:], compare_op=OP.not_equal,
            fill=0.0, base=0, pattern=[[-1, 128]], channel_multiplier=1)
        iota32 = const_pool.tile([128, C], F32)
        nc.gpsimd.iota(iota32[:], pattern=[[1, C]], base=0,
                       channel_multiplier=0,
                       allow_small_or_imprecise_dtypes=True)
        ones32 = const_pool.tile([C, 1], F32)
        nc.gpsimd.memset(ones32[:], 1.0)

        big_pool = ep(tc.tile_pool(name="big", bufs=1))
        fT = big_pool.tile([128, 2 * B], BF16)   # fT rows 0..127 | rows 128..255
        fT0 = fT[:, 0:B]
        fT1 = fT[:, B:2 * B]
        oh_sb = big_pool.tile([128, N_CHUNKS * C], BF16)   # onehot key blocks
        lab_sb = big_pool.tile([128, N_CHUNKS], F32)
        nc.sync.dma_start(lab_sb[:], lab.ap())

        # persistent PSUM accumulator for class sums
        cs_pool = ep(tc.tile_pool(name="csacc", bufs=1, space="PSUM"))
        cs_psum = cs_pool.tile([C, SHARD], F32)  # class_sum.T for anchors

        fin = ep(tc.tile_pool(name="fin", bufs=1))

        GRP = 8   # chunks per batched-sqrt group
        LAG = 7   # hot-loop key block emitted alongside stage-A chunk kb+LAG

        # CS matmuls run two key blocks behind the sim matmuls so the PE
        # never waits on the ACT exp of the current block.
        pending = []

        def cs_mms(okb, oe):
            for nb in range(2):
                nc.tensor.matmul(
                    cs_psum[:, nb * 512:(nb + 1) * 512],
                    oh_sb[:, okb * C:(okb + 1) * C],
                    oe[:, nb * 512:(nb + 1) * 512],
                    start=(okb == 0), stop=(okb == N_CHUNKS - 1))

        with tc.tile_pool(name="simp", bufs=2, space="PSUM") as simpool, \
             tc.tile_pool(name="esb", bufs=3) as epool:

            def hot_iter(kb):
                ps = simpool.tile([128, SHARD], F32, tag="ps")
                for kc, fTk in ((0, fT0), (1, fT1)):
                    for nb in range(2):
                        nc.tensor.matmul(
                            ps[:, nb * 512:(nb + 1) * 512],
                            fTk[:, kb * 128:(kb + 1) * 128],
                            fTk[:, nb * 512:(nb + 1) * 512],
                            start=(kc == 0), stop=(kc == 1))
                if len(pending) == 2:
                    cs_mms(*pending.pop(0))
                e = epool.tile([128, SHARD], BF16, tag="e")
                nc.scalar.activation(e[:], ps[:], ACT.Exp, scale=TEMP_INV)
                if kb < SHARD // 128:
                    # self-pairs: rotated key kb*128+p vs anchor col kb*128+p
                    nc.gpsimd.tensor_tensor(
                        e[:, kb * 128:(kb + 1) * 128],
                        e[:, kb * 128:(kb + 1) * 128], diagmask[:], OP.mult)
                pending.append((kb, e))

            # ---- stage A interleaved with the first hot-loop blocks ----
            # Stage A chunk ch feeds fT columns; hot block kb needs chunks
            # <= max(kb, 7), so kb = ch - LAG is safe and keeps the PE busy
            # on similarity matmuls while DMA/DVE/ACT run the next chunks.
            with tc.tile_pool(name="gacc", bufs=1, space="PSUM") as g_pool, \
                 tc.tile_pool(name="transp", bufs=1, space="PSUM") as tp_pool, \
                 tc.tile_pool(name="xchunk", bufs=GRP + 3) as xpool, \
                 tc.tile_pool(name="sq", bufs=2) as sqpool, \
                 tc.tile_pool(name="nrm", bufs=3) as npool:
                g_psum = g_pool.tile([C, H + 1], F32)  # onehot.T @ [f_norm|1]
                xs = {}
                for ch in range(N_CHUNKS):
                    x = xpool.tile([128, H], F32, tag="x")
                    xs[ch] = x
                    nc.sync.dma_start(
                        x[:], feat.ap()[ch * 128:(ch + 1) * 128, :])
                    sq = sqpool.tile([128, H], F32, tag="sq")
                    g = ch % GRP
                    if g == 0:
                        n2g = npool.tile([128, GRP], F32, tag="n2")
                        rig = npool.tile([128, GRP], F32, tag="ri")
                    nc.vector.scalar_tensor_tensor(
                        out=sq[:], in0=x[:], scalar=0.0, in1=x[:],
                        op0=OP.bypass, op1=OP.mult,
                        accum_out=n2g[:, g:g + 1])
                    if g == GRP - 1:
                        nrm = npool.tile([128, GRP], F32, tag="nrm")
                        nc.scalar.sqrt(nrm[:], n2g[:])
                        nc.vector.reciprocal(rig[:], nrm[:])
                        for j in range(GRP):
                            cj = ch - (GRP - 1) + j
                            xj = xs.pop(cj)
                            xb = xpool.tile([128, H + 1], BF16, tag="xb")
                            nc.vector.tensor_scalar_mul(
                                xb[:, 0:H], xj[:], rig[:, j:j + 1])
                            nc.gpsimd.memset(xb[:, H:H + 1], 1.0)
                            nc.gpsimd.tensor_scalar(
                                out=oh_sb[:, cj * C:(cj + 1) * C],
                                in0=iota32[:],
                                scalar1=lab_sb[:, cj:cj + 1], scalar2=None,
                                op0=OP.is_equal)
                            nc.tensor.matmul(
                                g_psum[:], oh_sb[:, cj * C:(cj + 1) * C],
                                xb[:], start=(cj == 0),
                                stop=(cj == N_CHUNKS - 1))
                            tp = tp_pool.tile([128, 256], BF16, tag="tp")
                            for kc in range(2):
                                nc.tensor.transpose(
                                    tp[:, kc * 128:(kc + 1) * 128],
                                    xb[:, kc * 128:(kc + 1) * 128],
                                    identity[:])
                            dst = fT[:].rearrange("p (k n) -> p k n", k=2)[
                                :, :, cj * 128:(cj + 1) * 128]
                            nc.vector.tensor_copy(
                                dst, tp[:].rearrange("p (k n) -> p k n", k=2))
                            if cj >= LAG:
                                hot_iter(cj - LAG)

                # ---- pre-tail finale work (independent of class sums) ----
                g_sb = fin.tile([C, H], BF16)
                nc.vector.tensor_copy(g_sb[:], g_psum[:, 0:H])
                counts = fin.tile([C, 1], F32)
                nc.vector.tensor_copy(counts[:], g_psum[:, H:H + 1])

                gT0 = fin.tile([128, C], BF16)
                gT1 = fin.tile([128, C], BF16)
                ohT = fin.tile([C, SHARD], F32)
                for kc, gTk in ((0, gT0), (1, gT1)):
                    tpg = tp_pool.tile([128, 256], BF16, tag="tp")
                    nc.tensor.transpose(
                        tpg[:, 0:C], g_sb[:, kc * 128:(kc + 1) * 128],
                        identity[0:C, 0:C])
                    nc.vector.tensor_copy(gTk[:], tpg[:, 0:C])
                for bkl in range(SHARD // 128):
                    tpo = tp_pool.tile([128, 256], BF16, tag="tp")
                    nc.tensor.transpose(
                        tpo[0:C, 0:128], oh_sb[:, bkl * C:(bkl + 1) * C],
                        identity[:])
                    nc.vector.tensor_copy(
                        ohT[:, bkl * 128:(bkl + 1) * 128], tpo[0:C, 0:128])

            with tc.tile_pool(name="rpsum", bufs=1, space="PSUM") as rpool:
                r_psum = rpool.tile([C, SHARD], F32)
                for kc, (gTk, fTk) in enumerate(((gT0, fT0), (gT1, fT1))):
                    for nb in range(2):
                        nc.tensor.matmul(
                            r_psum[:, nb * 512:(nb + 1) * 512], gTk[:],
                            fTk[:, nb * 512:(nb + 1) * 512],
                            start=(kc == 0), stop=(kc == 1))

                # counts_excl, masks, positives numerator / denominator
                ce = fin.tile([C, SHARD], F32)
                nc.vector.tensor_scalar(
                    out=ce[:], in0=ohT[:], scalar1=counts[:], scalar2=-1.0,
                    op0=OP.subtract, op1=OP.mult)
                mask = fin.tile([C, SHARD], F32)
                nc.vector.tensor_single_scalar(mask[:], ce[:], 0.5, OP.is_gt)
                ce1 = fin.tile([C, SHARD], F32)
                nc.vector.tensor_single_scalar(ce1[:], ce[:], 1.0, OP.max)
                rce = fin.tile([C, SHARD], F32)
                nc.vector.reciprocal(rce[:], ce1[:])
                nc.vector.tensor_tensor(rce[:], rce[:], mask[:], OP.mult)

                pnum = fin.tile([C, SHARD], F32)
                nc.vector.scalar_tensor_tensor(
                    out=pnum[:], in0=r_psum[:], scalar=1.0, in1=ohT[:],
                    op0=OP.subtract, op1=OP.mult)
                pden = fin.tile([C, SHARD], F32)
                nc.vector.tensor_tensor(pden[:], ohT[:], ce[:], OP.mult)

            with tc.tile_pool(name="rows_pre", bufs=1, space="PSUM") as rowp:
                prow = fin.tile([1, SHARD], F32)
                posrow = fin.tile([1, SHARD], F32)
                for src, dst in ((pden, prow), (pnum, posrow)):
                    row = rowp.tile([1, SHARD], F32, tag="row")
                    for nb in range(2):
                        nc.tensor.matmul(
                            row[:, nb * 512:(nb + 1) * 512], ones32[:],
                            src[:, nb * 512:(nb + 1) * 512],
                            start=True, stop=True)
                    nc.vector.tensor_copy(dst[:], row[:])

                valid = fin.tile([1, SHARD], F32)
                nc.vector.tensor_single_scalar(
                    valid[:], prow[:], 0.5, OP.is_gt)
                nc.vector.tensor_single_scalar(prow[:], prow[:], 1.0, OP.max)
                rp = fin.tile([1, SHARD], F32)
                nc.vector.reciprocal(rp[:], prow[:])
                mp = fin.tile([1, SHARD], F32)
                nc.vector.scalar_tensor_tensor(
                    out=mp[:], in0=posrow[:], scalar=TEMP_INV, in1=rp[:],
                    op0=OP.mult, op1=OP.mult)

            # ---- remaining hot-loop blocks ----
            for kb in range(N_CHUNKS - LAG, N_CHUNKS):
                hot_iter(kb)
            while pending:
                cs_mms(*pending.pop(0))

        # ---- post-hot finale: denominator path and output ----
        with tc.tile_pool(name="rows_post", bufs=1, space="PSUM") as rowpool:
            terms = fin.tile([C, SHARD], F32)
            nc.vector.tensor_tensor(terms[:], cs_psum[:], rce[:], OP.mult)
            logd = fin.tile([1, SHARD], F32)
            row = rowpool.tile([1, SHARD], F32)
            for nb in range(2):
                nc.tensor.matmul(
                    row[:, nb * 512:(nb + 1) * 512], ones32[:],
                    terms[:, nb * 512:(nb + 1) * 512],
                    start=True, stop=True)
            nc.vector.tensor_single_scalar(logd[:], row[:], 1e-30, OP.max)
            nc.scalar.activation(logd[:], logd[:], ACT.Ln)

            li = fin.tile([1, SHARD], F32)
            nc.vector.tensor_tensor(li[:], logd[:], mp[:], OP.subtract)
            nc.vector.tensor_tensor(li[:], li[:], valid[:], OP.mult)

            res = fin.tile([1, 2], F32)
            nc.vector.tensor_reduce(res[:, 0:1], li[:], axis=AX.X, op=OP.add)
            nc.vector.tensor_reduce(res[:, 1:2], valid[:], axis=AX.X, op=OP.add)
            nc.sync.dma_start(out.ap(), res[:])


_PROGRAM_CACHE = {}


def get_program(n_iters: int = 1):
    if n_iters not in _PROGRAM_CACHE:
        _PROGRAM_CACHE[n_iters] = build_program(n_iters)
    return _PROGRAM_CACHE[n_iters]


def make_in_maps(features: np.ndarray, labels: np.ndarray):
    features = np.ascontiguousarray(np.asarray(features, dtype=np.float32))
    labels_f = np.asarray(labels).astype(np.float32)
    in_maps = []
    for c in range(N_CORES):
        fr = np.roll(features, -c * SHARD, axis=0)
        lr = np.roll(labels_f, -c * SHARD)
        in_maps.append({
            "feat": np.ascontiguousarray(fr),
            "lab": np.ascontiguousarray(lr.reshape(N_CHUNKS, 128).T),
        })
    return in_maps


def kernel(features, labels):
    nc = get_program(1)
    in_maps = make_in_maps(features, labels)
    res = run_bass_kernel_spmd(nc, in_maps, list(range(N_CORES)))
    loss_sum = np.float32(0.0)
    n_valid = np.float32(0.0)
    for c in range(N_CORES):
        p = res.results[c]["partials"]
        loss_sum += np.float32(p[0, 0])
        n_valid += np.float32(p[0, 1])
    if n_valid > 0:
        loss = loss_sum / np.float32(max(n_valid, 1.0))
    else:
        loss = np.float32(0.0)
    return np.array(loss, dtype=np.float32)

